# revision 1
# baseline (speedup 1.0000x reference)
"""GQA kernel for Trainium2, 8 NeuronCores.

Problem: B=2, T=2048, HIDDEN=1024, 16 q-heads, 4 kv-heads, head_dim=64,
causal attention + output projection.

Sharding: core = (batch b = core//4, kv-group g = core%4). Each core handles
one batch element and the 4 query heads sharing kv-head g. o_proj is
column-parallel after per-chunk AllGathers (bf16) of the normalized attention
outputs within each batch group of 4 cores.

Device dataflow (all matmuls bf16 with fp32 PSUM accumulation):
  - host supplies xT = x[b].T in bf16 ([1024, 2048]; hidden on partitions)
  - qT/kT via W-stationary matmuls (outputs transposed: head_dim on partitions)
  - V natural via PE transposes of vT tiles; ones column appended -> softmax
    denominators fall out of the PV matmul for free
  - S^T = kT.T @ qT directly (no transposes in the attention inner loop);
    2 heads packed per pass via PE row-tiling (K=64 each)
  - exp on ACT engine; causal mask = bf16 0/1 multiply on diagonal tiles only
  - o^T_aug[65, Tq] accumulated per head in PSUM, evacuated to SBUF fast
    (frees PSUM for the next chunk); normalization trails off-path
  - per-chunk AllGather of normalized attn^T (bf16) -> col-parallel o_proj
  - output is outT [256, 2048] (columns 256g..256g+256 of out[b], transposed);
    host concatenates and transposes back.
"""

import sys

import numpy as np

try:
    import concourse.bass as bass
except ImportError:
    sys.path.insert(0, "/opt/trn_rl_repo")
    import concourse.bass as bass

import ml_dtypes
from contextlib import ExitStack

import concourse.tile as tile
from concourse import bacc, mybir
from concourse.bass import ds, ts
from concourse.bass_utils import run_bass_kernel_spmd
from concourse.masks import make_identity

BF16 = mybir.dt.bfloat16
F32 = mybir.dt.float32

P = 128
T = 2048
HID = 1024
KT = HID // P  # 8 k-tiles over hidden
CH = 512       # T_q chunk width
NCHUNK = T // CH
D = 64         # head dim
SCALE = D ** -0.5

_PROGRAM = None


def build_program():
    nc = bacc.Bacc(num_devices=8)

    xT_d = nc.declare_dram_parameter("xT", [HID, T], BF16, isOutput=False)
    wqkv_d = nc.declare_dram_parameter("wqkv", [HID, 384], BF16, isOutput=False)
    wo_d = nc.declare_dram_parameter("wo", [HID, 256], BF16, isOutput=False)
    mask_d = nc.declare_dram_parameter("maskc", [P, 1024], BF16, isOutput=False)
    outT_d = nc.declare_dram_parameter("outT", [256, T], F32, isOutput=True)

    with tile.TileContext(nc) as tc, ExitStack() as ctx:
        sing = ctx.enter_context(tc.tile_pool(name="sing", bufs=1))
        work = ctx.enter_context(tc.tile_pool(name="work", bufs=2, space="PSUM"))
        accp = ctx.enter_context(tc.tile_pool(name="accp", bufs=4, space="PSUM"))
        ptp = ctx.enter_context(tc.tile_pool(name="ptp", bufs=4))
        outp = ctx.enter_context(tc.tile_pool(name="outp", bufs=3))
        nrmp = ctx.enter_context(tc.tile_pool(name="nrmp", bufs=8))
        oevp = ctx.enter_context(tc.tile_pool(name="oevp", bufs=8))
        agp = ctx.enter_context(tc.tile_pool(name="agp", bufs=2))
        dram = ctx.enter_context(tc.tile_pool(name="dram", bufs=1, space="DRAM"))

        HT = T // 2
        ag_in = [
            dram.tile([256, HT], BF16, name=f"ag_in{i}") for i in range(2)
        ]
        ag_out = [
            dram.tile([4 * 256, HT], BF16, name=f"ag_out{i}") for i in range(2)
        ]

        # --- loads needed before chunk-0 compute: wqkv, xT chunk 0, mask ---
        wqkv_sb = sing.tile([P, KT, 384], BF16)
        nc.sync.dma_start(wqkv_sb, wqkv_d[:, :].rearrange("(kt p) n -> p kt n", p=P))
        xT_sb = sing.tile([P, KT, T], BF16)
        for kt in range(KT):
            nc.sync.dma_start(xT_sb[:, kt, ts(0, CH)], xT_d[ts(kt, P), ts(0, CH)])
        maskc = sing.tile([P, 1024], BF16)
        nc.sync.dma_start(maskc, mask_d[:, :])
        ident = sing.tile([P, P], BF16)
        make_identity(nc, ident)
        # --- deferred loads ---
        for c in range(1, NCHUNK):
            for kt in range(KT):
                nc.sync.dma_start(xT_sb[:, kt, ts(c, CH)], xT_d[ts(kt, P), ts(c, CH)])
        wo_sb = sing.tile([P, KT, 256], BF16)
        nc.sync.dma_start(wo_sb, wo_d[:, :].rearrange("(kt p) n -> p kt n", p=P))

        # blocks: 0 = qT heads (0,1); 1 = qT heads (2,3); 2 = [kT | vT]
        qkvT_sb = sing.tile([P, 3, T], BF16)
        kdup = sing.tile([P, T], BF16)        # kT duplicated on both partition halves
        vaug = sing.tile([P, 16, 66], BF16)   # V natural per T_k tile + ones col (64)
        nc.gpsimd.memset(vaug[:, :, 64:65], 1.0)
        agT = sing.tile([P, KT, T], BF16)     # gathered attn^T for o_proj

        for c in range(NCHUNK):
            cs = ts(c, CH)
            # ---- qkv projection for this T-chunk ----
            for blk in range(3):
                pj = work.tile([P, 1024], F32, tag="work", name=f"pj{c}_{blk}")
                for kt in range(KT):
                    nc.tensor.matmul(
                        pj[:, 0:CH],
                        wqkv_sb[:, kt, ts(blk, P)],
                        xT_sb[:, kt, cs],
                        start=(kt == 0),
                        stop=(kt == KT - 1),
                    )
                if blk < 2:
                    nc.vector.tensor_copy(qkvT_sb[:, blk, cs], pj[:, 0:CH])
                else:
                    nc.vector.tensor_copy(kdup[0:64, cs], pj[0:64, 0:CH])
                    nc.vector.tensor_copy(kdup[64:128, cs], pj[0:64, 0:CH])
                    nc.vector.tensor_copy(qkvT_sb[64:128, 2, cs], pj[64:128, 0:CH])

            # ---- V natural for the 4 new T_k tiles ----
            for j in range(4 * c, 4 * c + 4):
                vps = work.tile([P, 64], BF16, tag="work", name=f"vps{j}")
                nc.tensor.transpose(
                    vps[:, 0:64], qkvT_sb[64:128, 2, ts(j, P)], ident[64:128, 64:128]
                )
                nc.vector.tensor_copy(vaug[:, j, 0:64], vps[:, 0:64])

            # ---- attention for chunk c ----
            ntk = 4 * (c + 1)
            oa = [
                accp.tile([P, CH], F32, tag="acc", name=f"oa{c}_{h}")
                for h in range(4)
            ]
            for j in range(ntk):
                diag = j >= 4 * c
                d_off = P * (j - 4 * c)
                for hp in range(2):
                    s2 = work.tile([P, 1024], F32, tag="work", name=f"s2_{c}_{j}_{hp}")
                    nc.tensor.matmul(
                        s2[:, 0:CH],
                        kdup[0:64, ts(j, P)],
                        qkvT_sb[0:64, hp, cs],
                        start=True,
                        stop=True,
                        tile_position=(0, 0),
                    )
                    nc.tensor.matmul(
                        s2[:, CH:1024],
                        kdup[64:128, ts(j, P)],
                        qkvT_sb[64:128, hp, cs],
                        start=True,
                        stop=True,
                        tile_position=(64, 0),
                    )
                    pt = ptp.tile([P, 1024], BF16, tag="pt", name=f"pt{c}_{j}_{hp}")
                    nc.scalar.activation(pt, s2, mybir.ActivationFunctionType.Exp)
                    if diag:
                        msl = maskc[:, ds(384 - d_off, CH)]
                        nc.vector.tensor_mul(pt[:, 0:CH], pt[:, 0:CH], msl)
                        nc.vector.tensor_mul(pt[:, CH:1024], pt[:, CH:1024], msl)
                    for hh in range(2):
                        h = 2 * hp + hh
                        nc.tensor.matmul(
                            oa[h][0:65, :],
                            vaug[:, j, 0:65],
                            pt[:, ts(hh, CH)],
                            start=(j == 0),
                            stop=(j == ntk - 1),
                        )

            # ---- evacuate accumulators to SBUF fast (frees PSUM) ----
            oev = []
            for h in range(4):
                oe = oevp.tile([65, CH], F32, tag="oev", name=f"oev{c}_{h}")
                nc.vector.tensor_copy(oe, oa[h][0:65, :])
                oev.append(oe)

            # ---- normalize (off critical path) + stage + ship chunk ----
            atst = agp.tile([P, 2, CH], BF16, tag="atst", name=f"atst{c}")
            for h in range(4):
                recip = nrmp.tile([1, CH], F32, tag="rcp", name=f"rcp{c}_{h}")
                nc.vector.reciprocal(recip, oev[h][64:65, :])
                rb = nrmp.tile([64, CH], F32, tag="rb", name=f"rb{c}_{h}")
                nc.gpsimd.partition_broadcast(rb, recip)
                nc.vector.tensor_mul(
                    atst[ds(64 * (h % 2), 64), h // 2, :], oev[h][0:64, :], rb
                )
            half = c // 2
            agv = ag_in[half].rearrange("(blk p) t -> p blk t", p=P)
            nc.sync.dma_start(agv[:, :, ts(c % 2, CH)], atst)

            if c % 2 == 1:
                # ---- AllGather this half within the batch group ----
                # (half 0 hides under chunks 2-3's attention)
                nc.gpsimd.collective_compute(
                    "AllGather",
                    mybir.AluOpType.bypass,
                    replica_groups=[[0, 1, 2, 3], [4, 5, 6, 7]],
                    ins=[ag_in[half].opt()],
                    outs=[ag_out[half].opt()],
                )
                # ---- col-parallel o_proj for this half ----
                for kt in range(KT):
                    nc.sync.dma_start(
                        agT[:, kt, ts(half, HT)], ag_out[half][ts(kt, P), :]
                    )
                for c2 in (2 * half, 2 * half + 1):
                    for mb in range(2):
                        ps = work.tile(
                            [P, 1024], F32, tag="work", name=f"ps{c2}_{mb}"
                        )
                        for kt in range(KT):
                            nc.tensor.matmul(
                                ps[:, 0:CH],
                                wo_sb[:, kt, ts(mb, P)],
                                agT[:, kt, ts(c2, CH)],
                                start=(kt == 0),
                                stop=(kt == KT - 1),
                            )
                        ob = outp.tile([P, CH], F32, tag="ob", name=f"ob{c2}_{mb}")
                        nc.vector.tensor_copy(ob, ps[:, 0:CH])
                        nc.sync.dma_start(outT_d[ts(mb, P), ts(c2, CH)], ob)

    nc.finalize()
    return nc


def _prep_inputs(x, Wq, Wkv, Wo):
    bf = ml_dtypes.bfloat16
    x = np.asarray(x, dtype=np.float32)
    Wq = np.asarray(Wq, dtype=np.float32)
    Wkv = np.asarray(Wkv, dtype=np.float32)
    Wo = np.asarray(Wo, dtype=np.float32)

    # causal mask bank: M[r, m] = 1.0 iff r <= m - 384 (else 0)
    mask = (np.arange(P)[:, None] <= (np.arange(1024)[None, :] - 384)).astype(bf)

    xT = [np.ascontiguousarray(x[b].T).astype(bf) for b in range(2)]

    in_maps = []
    for core in range(8):
        b, g = core // 4, core % 4
        wq_g = Wq[:, 256 * g : 256 * (g + 1)] * SCALE
        wk_g = Wkv[:, 64 * g : 64 * (g + 1)]
        wv_g = Wkv[:, 256 + 64 * g : 256 + 64 * (g + 1)]
        wqkv = np.ascontiguousarray(
            np.concatenate([wq_g, wk_g, wv_g], axis=1)
        ).astype(bf)
        wo_g = np.ascontiguousarray(Wo[:, 256 * g : 256 * (g + 1)]).astype(bf)
        in_maps.append(
            {"xT": xT[b], "wqkv": wqkv, "wo": wo_g, "maskc": mask}
        )
    return in_maps


def run(x, Wq, Wkv, Wo, trace=False, **trace_kwargs):
    global _PROGRAM
    if _PROGRAM is None:
        _PROGRAM = build_program()
    nc = _PROGRAM
    in_maps = _prep_inputs(x, Wq, Wkv, Wo)
    res = run_bass_kernel_spmd(
        nc, in_maps, core_ids=list(range(8)), trace=trace, **trace_kwargs
    )
    outs = res.results
    full = np.empty((2, T, HID), dtype=np.float32)
    for b in range(2):
        outT_b = np.concatenate(
            [np.asarray(outs[4 * b + g]["outT"]) for g in range(4)], axis=0
        )  # [1024, 2048]
        full[b] = outT_b.T
    return full, res


def kernel(x, Wq, Wkv, Wo):
    out, _ = run(x, Wq, Wkv, Wo, trace=False)
    return out



# revision 4
# speedup vs baseline: 1.3881x; 1.3881x over previous
"""GQA kernel for Trainium2, 8 NeuronCores.

Problem: B=2, T=2048, HIDDEN=1024, 16 q-heads, 4 kv-heads, head_dim=64,
causal attention + output projection.

Sharding: core = (batch b = core//4, kv-group g = core%4). Each core handles
one batch element and the 4 query heads sharing kv-head g. o_proj is
column-parallel after per-chunk AllGathers (bf16) of the normalized attention
outputs within each batch group of 4 cores.

Device dataflow (all matmuls bf16 with fp32 PSUM accumulation):
  - host supplies xT = x[b].T in bf16 ([1024, 2048]; hidden on partitions)
  - qT/kT via W-stationary matmuls (outputs transposed: head_dim on partitions)
  - V natural via PE transposes of vT tiles; ones column appended -> softmax
    denominators fall out of the PV matmul for free
  - S^T = kT.T @ qT directly (no transposes in the attention inner loop);
    2 heads packed per pass via PE row-tiling (K=64 each)
  - exp on ACT engine; diagonal tiles restrict matmul/exp to the valid causal
    q-range and mask only the 128-wide triangle slab (bf16 0/1 multiply)
  - o^T_aug[65, Tq] accumulated per head in PSUM, evacuated to SBUF fast
    (frees PSUM for the next chunk); normalization trails off-path (batched
    approx reciprocal on DVE, broadcast on gpsimd)
  - per-chunk AllGather of normalized attn^T (bf16), issued as soon as each
    chunk is normalized so collectives overlap later chunks' attention
  - ALL o_proj matmuls are deferred until after the last attention chunk so
    the PE never queue-blocks on a collective; only the last chunk's
    AllGather tail is exposed (~o_proj of 3 chunks hides it)
  - output is outT [256, 2048] (columns 256g..256g+256 of out[b], transposed);
    host concatenates and transposes back.
"""

import sys

import numpy as np

try:
    import concourse.bass as bass
except ImportError:
    sys.path.insert(0, "/opt/trn_rl_repo")
    import concourse.bass as bass

import ml_dtypes
from contextlib import ExitStack

import concourse.tile as tile
from concourse import bacc, mybir
from concourse.bass import ds, ts
from concourse.bass_utils import run_bass_kernel_spmd
from concourse.masks import make_identity

BF16 = mybir.dt.bfloat16
F32 = mybir.dt.float32

P = 128
T = 2048
HID = 1024
KT = HID // P  # 8 k-tiles over hidden
CH = 512       # T_q chunk width
NCHUNK = T // CH
D = 64         # head dim
SCALE = D ** -0.5

_PROGRAM = None


def build_program():
    nc = bacc.Bacc(num_devices=8)

    xT_d = nc.declare_dram_parameter("xT", [HID, T], BF16, isOutput=False)
    wqkv_d = nc.declare_dram_parameter("wqkv", [HID, 384], BF16, isOutput=False)
    wo_d = nc.declare_dram_parameter("wo", [HID, 256], BF16, isOutput=False)
    mask_d = nc.declare_dram_parameter("maskc", [P, P], BF16, isOutput=False)
    outT_d = nc.declare_dram_parameter("outT", [256, T], F32, isOutput=True)

    with tile.TileContext(nc) as tc, ExitStack() as ctx:
        sing = ctx.enter_context(tc.tile_pool(name="sing", bufs=1))
        work = ctx.enter_context(tc.tile_pool(name="work", bufs=2, space="PSUM"))
        accp = ctx.enter_context(tc.tile_pool(name="accp", bufs=4, space="PSUM"))
        ptp = ctx.enter_context(tc.tile_pool(name="ptp", bufs=4))
        outp = ctx.enter_context(tc.tile_pool(name="outp", bufs=3))
        nrmp = ctx.enter_context(tc.tile_pool(name="nrmp", bufs=8))
        oevp = ctx.enter_context(tc.tile_pool(name="oevp", bufs=10))
        agp = ctx.enter_context(tc.tile_pool(name="agp", bufs=2))
        dram = ctx.enter_context(tc.tile_pool(name="dram", bufs=1, space="DRAM"))

        ag_in = [
            dram.tile([256, CH], BF16, name=f"ag_in{c}") for c in range(NCHUNK)
        ]
        ag_out = [
            dram.tile([4 * 256, CH], BF16, name=f"ag_out{c}")
            for c in range(NCHUNK)
        ]

        # --- loads needed before chunk-0 compute: wqkv, xT chunk 0, mask ---
        wqkv_sb = sing.tile([P, KT, 384], BF16)
        nc.sync.dma_start(wqkv_sb, wqkv_d[:, :].rearrange("(kt p) n -> p kt n", p=P))
        xT_sb = sing.tile([P, KT, T], BF16)
        for kt in range(KT):
            nc.sync.dma_start(xT_sb[:, kt, ts(0, CH)], xT_d[ts(kt, P), ts(0, CH)])
        maskc = sing.tile([P, P], BF16)
        nc.sync.dma_start(maskc, mask_d[:, :])
        ident = sing.tile([P, P], BF16)
        make_identity(nc, ident)
        # --- deferred loads ---
        for c in range(1, NCHUNK):
            for kt in range(KT):
                nc.sync.dma_start(xT_sb[:, kt, ts(c, CH)], xT_d[ts(kt, P), ts(c, CH)])
        wo_sb = sing.tile([P, KT, 256], BF16)
        nc.sync.dma_start(wo_sb, wo_d[:, :].rearrange("(kt p) n -> p kt n", p=P))

        # blocks: 0 = qT heads (0,1); 1 = qT heads (2,3); 2 = [kT | vT]
        qkvT_sb = sing.tile([P, 3, T], BF16)
        kdup = sing.tile([P, T], BF16)        # kT duplicated on both partition halves
        vaug = sing.tile([P, 16, 66], BF16)   # V natural per T_k tile + ones col (64)
        nc.gpsimd.memset(vaug[:, :, 64:65], 1.0)
        agT = sing.tile([P, KT, T], BF16)     # gathered attn^T for o_proj

        def qkv_proj(c):
            cs = ts(c, CH)
            for blk in range(3):
                pj = work.tile([P, 1024], F32, tag="work", name=f"pj{c}_{blk}")
                for kt in range(KT):
                    nc.tensor.matmul(
                        pj[:, 0:CH],
                        wqkv_sb[:, kt, ts(blk, P)],
                        xT_sb[:, kt, cs],
                        start=(kt == 0),
                        stop=(kt == KT - 1),
                    )
                if blk < 2:
                    nc.vector.tensor_copy(qkvT_sb[:, blk, cs], pj[:, 0:CH])
                else:
                    nc.vector.tensor_copy(kdup[0:64, cs], pj[0:64, 0:CH])
                    nc.vector.tensor_copy(kdup[64:128, cs], pj[0:64, 0:CH])
                    nc.vector.tensor_copy(qkvT_sb[64:128, 2, cs], pj[64:128, 0:CH])

        def v_nat(c):
            for j in range(4 * c, 4 * c + 4):
                vps = work.tile([P, 64], BF16, tag="work", name=f"vps{j}")
                nc.tensor.transpose(
                    vps[:, 0:64], qkvT_sb[64:128, 2, ts(j, P)], ident[64:128, 64:128]
                )
                nc.vector.tensor_copy(vaug[:, j, 0:64], vps[:, 0:64])

        def attn(c):
            ntk = 4 * (c + 1)
            oa = [
                accp.tile([P, CH], F32, tag="acc", name=f"oa{c}_{h}")
                for h in range(4)
            ]
            for j in range(ntk):
                r = j - 4 * c  # >= 0 on the block diagonal
                off = P * r if r >= 0 else 0
                w = CH - off
                for hp in range(2):
                    s2 = work.tile([P, 1024], F32, tag="work", name=f"s2_{c}_{j}_{hp}")
                    nc.tensor.matmul(
                        s2[:, ds(off, w)],
                        kdup[0:64, ts(j, P)],
                        qkvT_sb[0:64, hp, ds(CH * c + off, w)],
                        start=True,
                        stop=True,
                        tile_position=(0, 0),
                    )
                    nc.tensor.matmul(
                        s2[:, ds(CH + off, w)],
                        kdup[64:128, ts(j, P)],
                        qkvT_sb[64:128, hp, ds(CH * c + off, w)],
                        start=True,
                        stop=True,
                        tile_position=(64, 0),
                    )
                    pt = ptp.tile([P, 1024], BF16, tag="pt", name=f"pt{c}_{j}_{hp}")
                    if r >= 0:
                        for hh in range(2):
                            nc.scalar.activation(
                                pt[:, ds(CH * hh + off, w)],
                                s2[:, ds(CH * hh + off, w)],
                                mybir.ActivationFunctionType.Exp,
                            )
                        for hh in range(2):
                            nc.vector.tensor_mul(
                                pt[:, ds(CH * hh + off, P)],
                                pt[:, ds(CH * hh + off, P)],
                                maskc,
                            )
                    else:
                        nc.scalar.activation(
                            pt, s2, mybir.ActivationFunctionType.Exp
                        )
                    for hh in range(2):
                        h = 2 * hp + hh
                        nc.tensor.matmul(
                            oa[h][0:65, ds(off, w)],
                            vaug[:, j, 0:65],
                            pt[:, ds(CH * hh + off, w)],
                            start=(j == 0),
                            stop=(j == ntk - 1),
                            skip_group_check=True,
                        )
            return oa

        def oev_evac(c, oa):
            # fast PSUM evacuation: attention rows to oev tiles; the softmax
            # denominator row goes straight through an approx reciprocal
            # (recip is the last reader of each oa accumulator)
            oev, rcps = [], []
            for h in range(4):
                oe = oevp.tile([64, CH], F32, tag="oev", name=f"oev{c}_{h}")
                nc.vector.tensor_copy(oe, oa[h][0:64, :])
                oev.append(oe)
                dnm = nrmp.tile([1, CH], F32, tag="dnm", name=f"dnm{c}_{h}")
                nc.vector.tensor_copy(dnm, oa[h][64:65, :])
                rcp = nrmp.tile([1, CH], F32, tag="rcp", name=f"rcp{c}_{h}")
                nc.vector.reciprocal_approx_fast(rcp, dnm)
                rcps.append(rcp)
            return oev, rcps

        def epilogue(c, oev, rcps):
            # normalize (off critical path) + stage + AllGather this chunk
            atst = agp.tile([P, 2, CH], BF16, tag="atst", name=f"atst{c}")
            for h in range(4):
                rb = nrmp.tile([64, CH], F32, tag="rb", name=f"rb{c}_{h}")
                nc.gpsimd.partition_broadcast(rb, rcps[h])
                nc.vector.tensor_mul(
                    atst[ds(64 * (h % 2), 64), h // 2, :], oev[h], rb
                )
            agv = ag_in[c].rearrange("(blk p) t -> p blk t", p=P)
            nc.sync.dma_start(agv, atst)
            nc.gpsimd.collective_compute(
                "AllGather",
                mybir.AluOpType.bypass,
                replica_groups=[[0, 1, 2, 3], [4, 5, 6, 7]],
                ins=[ag_in[c].opt()],
                outs=[ag_out[c].opt()],
            )

        prev = None
        for c in range(NCHUNK):
            qkv_proj(c)
            if prev is not None:
                epilogue(c - 1, *prev)
            v_nat(c)
            oa = attn(c)
            prev = oev_evac(c, oa)
        epilogue(NCHUNK - 1, *prev)

        # ---- col-parallel o_proj, all chunks after attention ----
        def agT_load(c):
            for kt in range(KT):
                nc.sync.dma_start(
                    agT[:, kt, ts(c, CH)], ag_out[c][ts(kt, P), :]
                )

        for c in range(NCHUNK - 1):
            agT_load(c)
        for c in range(NCHUNK):
            if c == NCHUNK - 1:
                agT_load(c)
            for mb in range(2):
                ps = work.tile([P, 1024], F32, tag="work", name=f"ps{c}_{mb}")
                for kt in range(KT):
                    nc.tensor.matmul(
                        ps[:, 0:CH],
                        wo_sb[:, kt, ts(mb, P)],
                        agT[:, kt, ts(c, CH)],
                        start=(kt == 0),
                        stop=(kt == KT - 1),
                    )
                ob = outp.tile([P, CH], F32, tag="ob", name=f"ob{c}_{mb}")
                nc.vector.tensor_copy(ob, ps[:, 0:CH])
                nc.sync.dma_start(outT_d[ts(mb, P), ts(c, CH)], ob)

    nc.finalize()
    return nc


def _prep_inputs(x, Wq, Wkv, Wo):
    bf = ml_dtypes.bfloat16
    x = np.asarray(x, dtype=np.float32)
    Wq = np.asarray(Wq, dtype=np.float32)
    Wkv = np.asarray(Wkv, dtype=np.float32)
    Wo = np.asarray(Wo, dtype=np.float32)

    # lower-triangle mask: M[r, qi] = 1.0 iff r <= qi
    mask = (np.arange(P)[:, None] <= np.arange(P)[None, :]).astype(bf)

    xT = [np.ascontiguousarray(x[b].T).astype(bf) for b in range(2)]

    in_maps = []
    for core in range(8):
        b, g = core // 4, core % 4
        wq_g = Wq[:, 256 * g : 256 * (g + 1)] * SCALE
        wk_g = Wkv[:, 64 * g : 64 * (g + 1)]
        wv_g = Wkv[:, 256 + 64 * g : 256 + 64 * (g + 1)]
        wqkv = np.ascontiguousarray(
            np.concatenate([wq_g, wk_g, wv_g], axis=1)
        ).astype(bf)
        wo_g = np.ascontiguousarray(Wo[:, 256 * g : 256 * (g + 1)]).astype(bf)
        in_maps.append(
            {"xT": xT[b], "wqkv": wqkv, "wo": wo_g, "maskc": mask}
        )
    return in_maps


def run(x, Wq, Wkv, Wo, trace=False, **trace_kwargs):
    global _PROGRAM
    if _PROGRAM is None:
        _PROGRAM = build_program()
    nc = _PROGRAM
    in_maps = _prep_inputs(x, Wq, Wkv, Wo)
    res = run_bass_kernel_spmd(
        nc, in_maps, core_ids=list(range(8)), trace=trace, **trace_kwargs
    )
    outs = res.results
    full = np.empty((2, T, HID), dtype=np.float32)
    for b in range(2):
        outT_b = np.concatenate(
            [np.asarray(outs[4 * b + g]["outT"]) for g in range(4)], axis=0
        )  # [1024, 2048]
        full[b] = outT_b.T
    return full, res


def kernel(x, Wq, Wkv, Wo):
    out, _ = run(x, Wq, Wkv, Wo, trace=False)
    return out


# revision 6
# speedup vs baseline: 1.4197x; 1.0228x over previous
"""GQA kernel for Trainium2, 8 NeuronCores.

Problem: B=2, T=2048, HIDDEN=1024, 16 q-heads, 4 kv-heads, head_dim=64,
causal attention + output projection.

Sharding: core = (batch b = core//4, kv-group g = core%4). Each core handles
one batch element and the 4 query heads sharing kv-head g. o_proj is
ROW-parallel: each core contracts its own 4 heads (256 of 1024 attn dims)
against its 256 rows of Wo for ALL output columns, then a per-chunk bf16
ReduceScatter(add) across the batch group sums the partials and leaves each
core exactly its 256-column outT slice. No gather of attention outputs is
needed, so o_proj for chunk c runs immediately after chunk c is normalized
and overlaps the remaining attention chunks; only the last chunk's
ReduceScatter is tail-exposed.

Device dataflow (all matmuls bf16 with fp32 PSUM accumulation):
  - host supplies xT = x[b].T in bf16 ([1024, 2048]; hidden on partitions)
  - qT/kT via W-stationary matmuls (outputs transposed: head_dim on partitions)
  - V natural via PE transposes of vT tiles; ones column appended -> softmax
    denominators fall out of the PV matmul for free
  - S^T = kT.T @ qT directly (no transposes in the attention inner loop);
    2 heads packed per pass via PE row-tiling (K=64 each)
  - exp on ACT engine; diagonal tiles restrict matmul/exp to the valid causal
    q-range (single strided-AP exp across both packed heads) and mask only
    the 128-wide triangle slab (one bf16 0/1 multiply per head pair)
  - o^T_aug[65, Tq] accumulated per head in PSUM, evacuated to SBUF fast
    (frees PSUM for the next chunk); normalization trails off-path (approx
    reciprocal on DVE, broadcast on gpsimd)
  - per-chunk: partial o_proj (16 matmuls) -> bf16 staging -> DRAM ->
    ReduceScatter(add) into a Shared DRAM tile -> f32 upconvert to outT
  - output is outT [256, 2048] (rows 256g..256g+256 of out[b].T);
    host concatenates and transposes back.
"""

import sys

import numpy as np

try:
    import concourse.bass as bass
except ImportError:
    sys.path.insert(0, "/opt/trn_rl_repo")
    import concourse.bass as bass

import ml_dtypes
from contextlib import ExitStack

import concourse.tile as tile
from concourse import bacc, mybir
from concourse.bass import ds, ts
from concourse.bass_utils import run_bass_kernel_spmd
from concourse.masks import make_identity

BF16 = mybir.dt.bfloat16
F32 = mybir.dt.float32

P = 128
T = 2048
HID = 1024
KT = HID // P  # 8 k-tiles over hidden
CH = 512       # T_q chunk width
NCHUNK = T // CH
D = 64         # head dim
SCALE = D ** -0.5

_PROGRAM = None


def build_program():
    nc = bacc.Bacc(num_devices=8)

    xT_d = nc.declare_dram_parameter("xT", [HID, T], BF16, isOutput=False)
    wqkv_d = nc.declare_dram_parameter("wqkv", [HID, 384], BF16, isOutput=False)
    wo_d = nc.declare_dram_parameter("wo", [256, HID], BF16, isOutput=False)
    mask_d = nc.declare_dram_parameter("maskc", [P, 2 * P], BF16, isOutput=False)
    outT_d = nc.declare_dram_parameter("outT", [256, T], F32, isOutput=True)

    with tile.TileContext(nc) as tc, ExitStack() as ctx:
        sing = ctx.enter_context(tc.tile_pool(name="sing", bufs=1))
        work = ctx.enter_context(tc.tile_pool(name="work", bufs=2, space="PSUM"))
        accp = ctx.enter_context(tc.tile_pool(name="accp", bufs=4, space="PSUM"))
        ptp = ctx.enter_context(tc.tile_pool(name="ptp", bufs=4))
        outp = ctx.enter_context(tc.tile_pool(name="outp", bufs=3))
        nrmp = ctx.enter_context(tc.tile_pool(name="nrmp", bufs=8))
        oevp = ctx.enter_context(tc.tile_pool(name="oevp", bufs=10))
        agp = ctx.enter_context(tc.tile_pool(name="agp", bufs=2))
        psg = ctx.enter_context(tc.tile_pool(name="psg", bufs=2))
        dram = ctx.enter_context(tc.tile_pool(name="dram", bufs=1, space="DRAM"))

        partial_d = [
            dram.tile([HID, CH], BF16, name=f"partial{c}") for c in range(NCHUNK)
        ]
        rs_out = [
            dram.tile([256, CH], BF16, name=f"rs_out{c}")
            for c in range(NCHUNK)
        ]

        # --- loads needed before chunk-0 compute: wqkv, xT chunk 0, mask ---
        wqkv_sb = sing.tile([P, KT, 384], BF16)
        nc.sync.dma_start(wqkv_sb, wqkv_d[:, :].rearrange("(kt p) n -> p kt n", p=P))
        xT_sb = sing.tile([P, KT, T], BF16)
        for kt in range(KT):
            nc.sync.dma_start(xT_sb[:, kt, ts(0, CH)], xT_d[ts(kt, P), ts(0, CH)])
        maskc = sing.tile([P, 2, P], BF16)
        nc.sync.dma_start(maskc, mask_d[:, :].rearrange("p (b t) -> p b t", b=2))
        ident = sing.tile([P, P], BF16)
        make_identity(nc, ident)
        # --- deferred loads ---
        for c in range(1, NCHUNK):
            for kt in range(KT):
                nc.sync.dma_start(xT_sb[:, kt, ts(c, CH)], xT_d[ts(kt, P), ts(c, CH)])
        wo_sb = sing.tile([P, 2, HID], BF16)
        nc.sync.dma_start(wo_sb, wo_d[:, :].rearrange("(blk p) n -> p blk n", p=P))

        # blocks: 0 = qT heads (0,1); 1 = qT heads (2,3); 2 = [kT | vT]
        qkvT_sb = sing.tile([P, 3, T], BF16)
        kdup = sing.tile([P, T], BF16)        # kT duplicated on both partition halves
        vaug = sing.tile([P, 16, 66], BF16)   # V natural per T_k tile + ones col (64)
        nc.gpsimd.memset(vaug[:, :, 64:65], 1.0)

        def qkv_proj(c):
            cs = ts(c, CH)
            for blk in range(3):
                pj = work.tile([P, 1024], F32, tag="work", name=f"pj{c}_{blk}")
                for kt in range(KT):
                    nc.tensor.matmul(
                        pj[:, 0:CH],
                        wqkv_sb[:, kt, ts(blk, P)],
                        xT_sb[:, kt, cs],
                        start=(kt == 0),
                        stop=(kt == KT - 1),
                    )
                if blk < 2:
                    nc.vector.tensor_copy(qkvT_sb[:, blk, cs], pj[:, 0:CH])
                else:
                    nc.vector.tensor_copy(kdup[0:64, cs], pj[0:64, 0:CH])
                    nc.vector.tensor_copy(kdup[64:128, cs], pj[0:64, 0:CH])
                    nc.vector.tensor_copy(qkvT_sb[64:128, 2, cs], pj[64:128, 0:CH])

        def v_nat(c):
            for j in range(4 * c, 4 * c + 4):
                vps = work.tile([P, 64], BF16, tag="work", name=f"vps{j}")
                nc.tensor.transpose(
                    vps[:, 0:64], qkvT_sb[64:128, 2, ts(j, P)], ident[64:128, 64:128]
                )
                nc.vector.tensor_copy(vaug[:, j, 0:64], vps[:, 0:64])

        def attn(c):
            ntk = 4 * (c + 1)
            oa = [
                accp.tile([P, CH], F32, tag="acc", name=f"oa{c}_{h}")
                for h in range(4)
            ]
            for j in range(ntk):
                r = j - 4 * c  # >= 0 on the block diagonal
                off = P * r if r >= 0 else 0
                w = CH - off
                for hp in range(2):
                    s2 = work.tile([P, 1024], F32, tag="work", name=f"s2_{c}_{j}_{hp}")
                    nc.tensor.matmul(
                        s2[:, ds(off, w)],
                        kdup[0:64, ts(j, P)],
                        qkvT_sb[0:64, hp, ds(CH * c + off, w)],
                        start=True,
                        stop=True,
                        tile_position=(0, 0),
                    )
                    nc.tensor.matmul(
                        s2[:, ds(CH + off, w)],
                        kdup[64:128, ts(j, P)],
                        qkvT_sb[64:128, hp, ds(CH * c + off, w)],
                        start=True,
                        stop=True,
                        tile_position=(64, 0),
                    )
                    pt = ptp.tile([P, 1024], BF16, tag="pt", name=f"pt{c}_{j}_{hp}")
                    if r >= 0:
                        s2v = s2.rearrange("p (b t) -> p b t", b=2)
                        ptv = pt.rearrange("p (b t) -> p b t", b=2)
                        nc.scalar.activation(
                            ptv[:, :, ds(off, w)],
                            s2v[:, :, ds(off, w)],
                            mybir.ActivationFunctionType.Exp,
                        )
                        nc.vector.tensor_mul(
                            ptv[:, :, ds(off, P)], ptv[:, :, ds(off, P)], maskc
                        )
                    else:
                        nc.scalar.activation(
                            pt, s2, mybir.ActivationFunctionType.Exp
                        )
                    for hh in range(2):
                        h = 2 * hp + hh
                        nc.tensor.matmul(
                            oa[h][0:65, ds(off, w)],
                            vaug[:, j, 0:65],
                            pt[:, ds(CH * hh + off, w)],
                            start=(j == 0),
                            stop=(j == ntk - 1),
                            skip_group_check=True,
                        )
            return oa

        def oev_evac(c, oa):
            # fast PSUM evacuation: attention rows to oev tiles; the softmax
            # denominator row goes straight through an approx reciprocal
            oev, rcps = [], []
            for h in range(4):
                oe = oevp.tile([64, CH], F32, tag="oev", name=f"oev{c}_{h}")
                nc.vector.tensor_copy(oe, oa[h][0:64, :])
                oev.append(oe)
                dnm = nrmp.tile([1, CH], F32, tag="dnm", name=f"dnm{c}_{h}")
                nc.vector.tensor_copy(dnm, oa[h][64:65, :])
                rcp = nrmp.tile([1, CH], F32, tag="rcp", name=f"rcp{c}_{h}")
                nc.vector.reciprocal_approx_fast(rcp, dnm)
                rcps.append(rcp)
            return oev, rcps

        def epilogue(c, oev, rcps):
            # normalize (off critical path): atst[:, blk, :] holds this
            # core's 4 heads' normalized attn^T for the chunk (bf16)
            atst = agp.tile([P, 2, CH], BF16, tag="atst", name=f"atst{c}")
            for h in range(4):
                rb = nrmp.tile([64, CH], F32, tag="rb", name=f"rb{c}_{h}")
                nc.gpsimd.partition_broadcast(rb, rcps[h])
                nc.vector.tensor_mul(
                    atst[ds(64 * (h % 2), 64), h // 2, :], oev[h], rb
                )
            return atst

        def o_proj(c, atst):
            # row-parallel partial o_proj: all 1024 output dims from this
            # core's 256 attn dims; bf16 partials staged to DRAM, then
            # ReduceScatter(add) within the batch group
            pstage = psg.tile([P, KT, CH], BF16, tag="pstage", name=f"pstage{c}")
            for mb in range(KT):
                ps = work.tile([P, 1024], F32, tag="work", name=f"ps{c}_{mb}")
                for blk in range(2):
                    nc.tensor.matmul(
                        ps[:, 0:CH],
                        wo_sb[:, blk, ts(mb, P)],
                        atst[:, blk, :],
                        start=(blk == 0),
                        stop=(blk == 1),
                    )
                nc.vector.tensor_copy(pstage[:, mb, :], ps[:, 0:CH])
            nc.sync.dma_start(
                partial_d[c].rearrange("(mb p) t -> p mb t", p=P), pstage
            )
            nc.gpsimd.collective_compute(
                "ReduceScatter",
                mybir.AluOpType.add,
                replica_groups=[[0, 1, 2, 3], [4, 5, 6, 7]],
                ins=[partial_d[c].opt()],
                outs=[rs_out[c].opt()],
            )

        prev = None
        for c in range(NCHUNK):
            qkv_proj(c)
            if prev is not None:
                atst = epilogue(c - 1, *prev)
                o_proj(c - 1, atst)
            v_nat(c)
            oa = attn(c)
            prev = oev_evac(c, oa)
        atst = epilogue(NCHUNK - 1, *prev)
        o_proj(NCHUNK - 1, atst)

        # ---- upconvert ReduceScatter results to f32 outT ----
        for c in range(NCHUNK):
            rsb = outp.tile([P, 2, CH], BF16, tag="rsb", name=f"rsb{c}")
            nc.sync.dma_start(
                rsb, rs_out[c].rearrange("(blk p) t -> p blk t", p=P)
            )
            osb = outp.tile([P, 2, CH], F32, tag="osb", name=f"osb{c}")
            nc.vector.tensor_copy(osb, rsb)
            nc.sync.dma_start(
                outT_d[:, ts(c, CH)].rearrange("(blk p) t -> p blk t", p=P), osb
            )

    nc.finalize()
    return nc


def _prep_inputs(x, Wq, Wkv, Wo):
    bf = ml_dtypes.bfloat16
    x = np.asarray(x, dtype=np.float32)
    Wq = np.asarray(Wq, dtype=np.float32)
    Wkv = np.asarray(Wkv, dtype=np.float32)
    Wo = np.asarray(Wo, dtype=np.float32)

    # lower-triangle mask, duplicated for the two packed heads:
    # M[r, b, qi] = 1.0 iff r <= qi
    tri = (np.arange(P)[:, None] <= np.arange(P)[None, :])
    mask = np.concatenate([tri, tri], axis=1).astype(bf)

    xT = [np.ascontiguousarray(x[b].T).astype(bf) for b in range(2)]

    in_maps = []
    for core in range(8):
        b, g = core // 4, core % 4
        wq_g = Wq[:, 256 * g : 256 * (g + 1)] * SCALE
        wk_g = Wkv[:, 64 * g : 64 * (g + 1)]
        wv_g = Wkv[:, 256 + 64 * g : 256 + 64 * (g + 1)]
        wqkv = np.ascontiguousarray(
            np.concatenate([wq_g, wk_g, wv_g], axis=1)
        ).astype(bf)
        wo_g = np.ascontiguousarray(Wo[256 * g : 256 * (g + 1), :]).astype(bf)
        in_maps.append(
            {"xT": xT[b], "wqkv": wqkv, "wo": wo_g, "maskc": mask}
        )
    return in_maps


def run(x, Wq, Wkv, Wo, trace=False, **trace_kwargs):
    global _PROGRAM
    if _PROGRAM is None:
        _PROGRAM = build_program()
    nc = _PROGRAM
    in_maps = _prep_inputs(x, Wq, Wkv, Wo)
    res = run_bass_kernel_spmd(
        nc, in_maps, core_ids=list(range(8)), trace=trace, **trace_kwargs
    )
    outs = res.results
    full = np.empty((2, T, HID), dtype=np.float32)
    for b in range(2):
        outT_b = np.concatenate(
            [np.asarray(outs[4 * b + g]["outT"]) for g in range(4)], axis=0
        )  # [1024, 2048]
        full[b] = outT_b.T
    return full, res


def kernel(x, Wq, Wkv, Wo):
    out, _ = run(x, Wq, Wkv, Wo, trace=False)
    return out
